# revision 21
# baseline (speedup 1.0000x reference)
"""TRN2 Bass kernel for nn_KVGather: out[b,i,t] = kv[b, r_idx[b,i,t]] * r_weight[b,i,t].

Full shapes: r_idx/r_weight (32,49,4), kv (32,49,64,256) f32 -> out (32,49,4,64,256) f32.

Sharding: batch dim n=32 across 8 cores (4 batches/core), pure data parallel.

Per-core device kernel (memory-bound; ~59 MB HBM traffic/core, ~143 us
output-DMA floor at 360 GB/s):
  - The gather is a one-hot matmul on the (otherwise idle) PE array: the
    host builds, per pair of batches (K=98 kv rows) and per group of 128
    output tiles, a one-hot bf16 stationary matrix S[98, 128] with exact-1.0
    entries; psum[m, 0:512] = sum_r S[r, m] * kv[r, chunk] = kv[row(m), chunk].
    Fully static program - no dynamic-offset register loads (the original
    design spent ~270 us in TENSOR_LOAD sequencer stalls on DVE/ACT).
  - kv is cast to bf16 on host (halves input traffic; one-hot entries are
    exact so rel err = bf16(kv) rounding ~2^-9, gate is 2e-2).
  - Exact f32 weights applied at the PSUM->SBUF drain (tensor_scalar /
    activation-Copy-scale with per-partition scalar), split greedily
    across DVE / ACT (GPSIMD cannot read PSUM on TRN2).
  - Stage layout [128 tiles (partition) x 8192 f32]: output DMA descriptors
    are 32 KB DRAM-contiguous, 128 per DMA = 8 per DMA engine (16 engines).
    Output DMAs exclusively on the SP queue (in produce order - anything
    else head-blocks the FIFO); inputs on the ACT/DVE HWDGE queues.
  - 392 tiles/pair = 3 groups of 128 + 8 leftover. A matmul streams all 512
    moving cols regardless of M, so an M=8 remainder group would waste
    ~17 us of PE per pair (PE at 256 matmuls was co-critical with the
    output DMA). Instead the host pre-gathers the 16 leftover rows into a
    small extra input in gather-free layout ([128p, tile*128]) and they are
    scaled directly by DVE/ACT - PE does only 192 uniform matmuls (~103 us).
  - kv loads are split per column-half and spread over the SP + Pool queues
    so the first matmul can start after ~5 us instead of ~25 us.
"""

import os
import sys

sys.path.insert(0, "/opt/trn_rl_repo")

import numpy as np

N, P2, TOPK, HW_KV, C_KV = 32, 49, 4, 64, 256
NCORES = 8
NB = N // NCORES  # batches per core
ROW_ELEMS = HW_KV * C_KV  # 16384 f32 per kv row / output tile
TILES = NB * P2 * TOPK  # 784 output tiles per core
PAIRS = NB // 2  # 2 batch-pairs per core
KPAIR = 2 * P2  # 98 kv rows per pair (contraction dim)
TILES_PER_PAIR = 2 * P2 * TOPK  # 392
NGROUP = 3  # full groups of 128 tiles per pair
REM = TILES_PER_PAIR - NGROUP * 128  # 8 leftover tiles per pair
NREM = PAIRS * REM  # 16 leftover tiles per core
CHUNK = 512  # matmul moving free dim (= PSUM bank)
SUB = 8192  # f32 per output sub-stage (32 KB descriptors)
NSUB = ROW_ELEMS // SUB  # 2
CPS = SUB // CHUNK  # 16 chunks per sub
CREM = ROW_ELEMS // 128  # 128 f32 per partition in gather-free layout

_compiled = None


def _build():
    import concourse.bass as bass  # noqa: F401
    import concourse.tile as tile
    from concourse import bacc, mybir

    nc = bacc.Bacc("TRN2", target_bir_lowering=False, debug=False)

    f32 = mybir.dt.float32
    bf16 = mybir.dt.bfloat16

    kv_d = nc.dram_tensor(
        "kv", [PAIRS, KPAIR, ROW_ELEMS], bf16, kind="ExternalInput"
    ).ap()
    stat_d = nc.dram_tensor(
        "stat", [KPAIR, PAIRS * NGROUP * 128], bf16, kind="ExternalInput"
    ).ap()
    w_d = nc.dram_tensor("w", [128, PAIRS * NGROUP], f32, kind="ExternalInput").ap()
    rem_d = nc.dram_tensor("rem", [128, NREM * CREM], bf16, kind="ExternalInput").ap()
    wrem_d = nc.dram_tensor("wrem", [128, NREM], f32, kind="ExternalInput").ap()
    out_d = nc.dram_tensor("out", [TILES, ROW_ELEMS], f32, kind="ExternalOutput").ap()

    COPY = mybir.ActivationFunctionType.Copy
    MULT = mybir.AluOpType.mult
    HALF = ROW_ELEMS // 2

    with tile.TileContext(nc) as tc:
        with (
            tc.tile_pool(name="const", bufs=1) as cpool,
            tc.tile_pool(name="kvp", bufs=2) as kvpool,
            tc.tile_pool(name="stage", bufs=4) as spool,
            tc.psum_pool(name="ps", bufs=4) as ppool,
        ):
            stat_sb = cpool.tile([KPAIR, PAIRS * NGROUP * 128], bf16, tag="stat")
            w_sb = cpool.tile([128, PAIRS * NGROUP], f32, tag="w")
            rem_sb = cpool.tile([128, NREM * CREM], bf16, tag="rem")
            wrem_sb = cpool.tile([128, NREM], f32, tag="wrem")
            rem_st = cpool.tile([128, NREM * CREM], f32, tag="rem_st")

            kv_sb = [
                kvpool.tile([KPAIR, ROW_ELEMS], bf16, tag="kv", name=f"kv{p}")
                for p in range(PAIRS)
            ]

            # pair-0 kv on the (empty) SP queue, split per column-half so the
            # first matmul only waits for the first half; pair-1 + rem inputs
            # on the ACT/DVE HWDGE queues (SWDGE serializes ~4us per DMA on
            # the Pool engine and ran at ~113 GB/s)
            # kv0 split into quarters across the SP + ACT queues so the first
            # matmul starts ~3us after the preamble instead of ~14us
            Q = ROW_ELEMS // 4
            nc.scalar.dma_start(stat_sb[:], stat_d[:])
            nc.sync.dma_start(kv_sb[0][:, 0:Q], kv_d[0][:, 0:Q])
            nc.scalar.dma_start(kv_sb[0][:, Q : 2 * Q], kv_d[0][:, Q : 2 * Q])
            nc.scalar.dma_start(kv_sb[0][:, 2 * Q : 3 * Q], kv_d[0][:, 2 * Q : 3 * Q])
            nc.scalar.dma_start(kv_sb[0][:, 3 * Q :], kv_d[0][:, 3 * Q :])
            nc.scalar.dma_start(w_sb[:], w_d[:])
            nc.scalar.dma_start(kv_sb[1][:, 0:HALF], kv_d[1][:, 0:HALF])
            nc.scalar.dma_start(kv_sb[1][:, HALF:], kv_d[1][:, HALF:])
            nc.gpsimd.dma_start(rem_sb[:], rem_d[:])
            nc.gpsimd.dma_start(wrem_sb[:], wrem_d[:])

            # greedy engine assignment for drains (ns per op, measured)
            load = {"dve": 0.0, "act": 0.0}
            cost_drain = {"dve": 1310.0, "act": 1500.0}
            cost_rem = {"dve": 330.0, "act": 400.0}

            def pick(cost):
                e = min(load, key=lambda e: load[e] + cost[e])
                load[e] += cost[e]
                return e

            def do_rem():
                # leftover tiles: static scale from gather-free layout
                for k in range(NREM):
                    src = rem_sb[:, k * CREM : (k + 1) * CREM]
                    dst = rem_st[:, k * CREM : (k + 1) * CREM]
                    wap = wrem_sb[:, k : k + 1]
                    if pick(cost_rem) == "dve":
                        nc.vector.tensor_scalar(dst, src, wap, None, MULT)
                    else:
                        nc.scalar.activation(dst, src, COPY, scale=wap)
                # on the ACT queue: the SP output FIFO must never head-block
                # on rem_st readiness
                for p in range(PAIRS):
                    r0 = p * TILES_PER_PAIR + NGROUP * 128
                    dst = out_d[r0 : r0 + REM, :].rearrange(
                        "j (p c) -> p j c", p=128
                    )
                    src = rem_st[:, p * REM * CREM : (p + 1) * REM * CREM].rearrange(
                        "p (j c) -> p j c", c=CREM
                    )
                    nc.scalar.dma_start(dst, src)

            for p in range(PAIRS):
                for g in range(NGROUP):
                    u = p * NGROUP + g
                    lhsT = stat_sb[:, u * 128 : (u + 1) * 128]
                    wap = w_sb[:, u : u + 1]
                    j0 = p * TILES_PER_PAIR + g * 128
                    for sub in range(NSUB):
                        st = spool.tile([128, SUB], f32, tag="st")
                        # 2-bank PSUM tiles: two matmuls, one 1024-wide drain
                        # (halves per-drain fixed overhead; drains were
                        # co-critical with the output DMA at 512 wide)
                        for d8 in range(CPS // 2):
                            c = sub * CPS + d8 * 2
                            ps = ppool.tile([128, 2 * CHUNK], f32, tag="ps")
                            for h in range(2):
                                nc.tensor.matmul(
                                    ps[:, h * CHUNK : (h + 1) * CHUNK],
                                    lhsT,
                                    kv_sb[p][:, (c + h) * CHUNK : (c + h + 1) * CHUNK],
                                    start=True,
                                    stop=True,
                                )
                            dst = st[:, d8 * 2 * CHUNK : (d8 + 1) * 2 * CHUNK]
                            if pick(cost_drain) == "dve":
                                nc.vector.tensor_scalar(dst, ps[:], wap, None, MULT)
                            else:
                                nc.scalar.activation(dst, ps[:], COPY, scale=wap)
                            if p == 0 and g == 0 and sub == 0 and d8 == 3:
                                # half-size first DMA so the output queue
                                # starts ~4us earlier
                                nc.sync.dma_start(
                                    out_d[j0 : j0 + 128, 0 : SUB // 2],
                                    st[:, 0 : SUB // 2],
                                )
                        if p == 0 and g == 0 and sub == 0:
                            nc.sync.dma_start(
                                out_d[j0 : j0 + 128, SUB // 2 : SUB],
                                st[:, SUB // 2 :],
                            )
                        else:
                            nc.sync.dma_start(
                                out_d[j0 : j0 + 128, sub * SUB : (sub + 1) * SUB],
                                st[:],
                            )
                        if p == 0 and g == 1 and sub == 1:
                            do_rem()

    nc.compile()
    return nc


def _get_compiled():
    global _compiled
    if _compiled is None:
        _compiled = _build()
    return _compiled


def _enable_trace_hook():
    """Register the axon NTFF profile hook (missing antenv.axon_hooks shim)."""
    import types

    try:
        import antenv.axon_hooks  # noqa: F401

        return
    except ImportError:
        pass
    try:
        import antenv

        mod = types.ModuleType("antenv.axon_hooks")
        holder = {}
        mod.set_axon_ntff_profile_hook = lambda h: holder.__setitem__("h", h)
        mod.get_axon_ntff_profile_hook = lambda: holder.get("h")
        antenv.axon_hooks = mod
        sys.modules["antenv.axon_hooks"] = mod
        if "/root/.axon_site" not in sys.path:
            sys.path.insert(0, "/root/.axon_site")
        from trn_agent_boot.trn_boot import _ntff_profile_via_ctypes

        mod.set_axon_ntff_profile_hook(
            _ntff_profile_via_ctypes("/opt/axon/libaxon_pjrt.so")
        )

        import concourse.bass_utils as bu

        orig = bu.upload_artifacts

        def _safe_upload(tmpdir):
            try:
                return orig(tmpdir)
            except Exception:
                return tmpdir

        bu.upload_artifacts = _safe_upload
    except Exception as e:  # tracing is best-effort
        print(f"trace hook setup failed: {e}")


def kernel(r_idx, r_weight, kv):
    import ml_dtypes
    from concourse.bass_utils import run_bass_kernel_spmd

    bf16 = ml_dtypes.bfloat16

    r_idx = np.asarray(r_idx)
    r_weight = np.asarray(r_weight, dtype=np.float32)
    kv = np.ascontiguousarray(np.asarray(kv, dtype=np.float32))
    assert r_idx.shape == (N, P2, TOPK) and kv.shape == (N, P2, HW_KV, C_KV)

    nc = _get_compiled()

    nfull = NGROUP * 128  # 384 full-group tiles per pair
    jlv = np.arange(nfull)
    gv = jlv // 128  # group 0..2
    mv = jlv % 128  # position within group
    b_in = jlv // (P2 * TOPK)  # batch within pair (0/1)
    rem = jlv % (P2 * TOPK)  # within-batch tile index (i*TOPK + t)
    jrem = np.arange(nfull, TILES_PER_PAIR)  # leftover tiles
    rb_in = jrem // (P2 * TOPK)
    rrem = jrem % (P2 * TOPK)

    in_maps = []
    for cidx in range(NCORES):
        b0 = cidx * NB
        kvf = kv[b0 : b0 + NB].reshape(PAIRS, KPAIR, ROW_ELEMS)
        kvs = np.ascontiguousarray(kvf.astype(bf16))
        idx = r_idx[b0 : b0 + NB].reshape(NB, P2 * TOPK).astype(np.int64)
        wgt = r_weight[b0 : b0 + NB].reshape(NB, P2 * TOPK)
        stat = np.zeros((KPAIR, PAIRS * NGROUP, 128), dtype=bf16)
        w = np.zeros((128, PAIRS * NGROUP), dtype=np.float32)
        remv = np.empty((PAIRS, REM, 128, CREM), dtype=bf16)
        wrem = np.empty((128, PAIRS, REM), dtype=np.float32)
        for p in range(PAIRS):
            b = 2 * p + b_in
            rr = b_in * P2 + idx[b, rem]
            u = p * NGROUP + gv
            stat[rr, u, mv] = 1.0
            w[mv, u] = wgt[b, rem]
            rbl = 2 * p + rb_in
            rrows = rb_in * P2 + idx[rbl, rrem]
            remv[p] = kvs[p, rrows].reshape(REM, 128, CREM)
            wrem[:, p, :] = wgt[rbl, rrem][None, :]
        in_maps.append(
            {
                "kv": kvs,
                "stat": np.ascontiguousarray(stat.reshape(KPAIR, -1)),
                "w": w,
                "rem": np.ascontiguousarray(
                    remv.transpose(2, 0, 1, 3).reshape(128, NREM * CREM)
                ),
                "wrem": np.ascontiguousarray(wrem.reshape(128, NREM)),
            }
        )

    trace = bool(int(os.environ.get("KV_TRACE", "0")))
    if trace:
        _enable_trace_hook()
    res = run_bass_kernel_spmd(nc, in_maps, list(range(NCORES)), trace=trace)

    if trace:
        kernel.last_exec_time_ns = res.exec_time_ns
        kernel.last_trace = (
            res.instructions_and_trace[1] if res.instructions_and_trace else None
        )
        kernel.last_profile_json = getattr(res, "profile_json", None)

    out = np.empty((N, P2, TOPK, HW_KV, C_KV), dtype=np.float32)
    for c in range(NCORES):
        b0 = c * NB
        out[b0 : b0 + NB] = res.results[c]["out"].reshape(NB, P2, TOPK, HW_KV, C_KV)
    return out
